# revision 21
# baseline (speedup 1.0000x reference)
"""Bahdanau additive attention on 8 TRN2 NeuronCores (fp8 DoubleRow rev).

Problem shapes: encoder_hiddens [16, 4096, 1024] f32, decoder_hidden [16, 1024],
We [1024, 512], be [512], Wd [1024, 512], bd [512], Wo [512, 1], bo [1].
Output: context [16, 1024] f32.

Sharding: data-parallel over batch (2 batches per core). Host prep stages the
encoder twice in chunk-contiguous SBUF-ready layouts:
  - bf16 copy [B_loc, 8pr, 128p, KO=8, 1024s] for the context weighted sum
  - fp8 e4m3 copy [B_loc, 8pr, 128p, 2h, SKO=4, 2, 512s] (DoubleRow k-pairing)
and precomputes the decoder projection bias (dec@Wd + be + bd) on host, so the
on-chip kernel is only: enc_proj (fp8 DoubleRow matmuls, We pre-scaled x64),
tanh (ACT, scale=1/64 fused), score matmul (bf16), exp (ACT, l via accum),
context accumulation (scalar_tensor_tensor on DVE; 2 of 8 ko-chunks go
tensor_tensor on DVE + Copy-accum reduce on ACT to balance engine load).
The softmax division happens on host (ctx and l are shipped out together).

Work is organized in 1024-wide pairs (two 512 matmul chunks) to halve DVE/ACT
instruction overheads. Per pair: 32 DoubleRow MMs (4 m x 4 k x 2 halves,
k-major so the stationary weights reload once per (m,k)) + 8 bf16 score MMs;
tanh is 1024-wide over paired PSUM tiles. Final pair's epilogue runs in
width-256 quarters to pipeline the tail.
"""

import numpy as np

B, S, E_ENC, E_DEC, H = 16, 4096, 1024, 1024, 512
NCORES = 8
B_LOC = B // NCORES   # 2
KO = E_ENC // 128     # 8 bf16 e-chunks
SKO = E_ENC // 256    # 4 fp8 DoubleRow super-k steps
MH = H // 128         # 4 h-chunks
WSCALE = 64.0         # We pre-scale for fp8 dynamic range (undone in tanh)
SP = 1024             # pair width
ACT_KOS = 2           # ko-chunks whose reduce runs on ACT instead of DVE


def build_graph(n_warm=18, n_tail_parts=4):
    import concourse.bacc as bacc
    import concourse.mybir as mybir
    import concourse.tile as tile

    f32 = mybir.dt.float32
    bf16 = mybir.dt.bfloat16
    f8 = mybir.dt.float8e4
    DR = mybir.MatmulPerfMode.DoubleRow
    AF = mybir.ActivationFunctionType
    ALU = mybir.AluOpType
    X = mybir.AxisListType.X

    n_pairs = S // SP                       # per batch (4)
    n_idx = B_LOC * n_pairs + (n_tail_parts - 1)

    nc = bacc.Bacc()
    enc8_d = nc.declare_dram_parameter(
        "enc8", [B_LOC, n_pairs, 128, 2, SKO, 2, 512], f8, isOutput=False
    )
    # encb split in ko halves so the two streams ride different DMA queues
    enclo_d = nc.declare_dram_parameter(
        "enclo", [B_LOC, n_pairs, 128, KO // 2, SP], bf16, isOutput=False
    )
    enchi_d = nc.declare_dram_parameter(
        "enchi", [B_LOC, n_pairs, 128, KO // 2, SP], bf16, isOutput=False
    )
    we8_d = nc.declare_dram_parameter("we8", [128, SKO, 2, MH, 128], f8, isOutput=False)
    wo_d = nc.declare_dram_parameter("wo", [128, MH, 128], bf16, isOutput=False)
    bias_d = nc.declare_dram_parameter("bias", [128, MH, B_LOC], f32, isOutput=False)
    # out: [:, b, :KO] = unnormalized ctx, [:, b, KO] = softmax denominator l
    out_d = nc.declare_dram_parameter("out", [128, B_LOC, KO + 1], f32, isOutput=True)

    with tile.TileContext(nc) as tc:
        with (
            tc.tile_pool(name="const", bufs=1) as const,
            tc.tile_pool(name="enc8", bufs=3) as enc8_pool,
            tc.tile_pool(name="encb", bufs=4) as encb_pool,
            tc.tile_pool(name="ep", bufs=2, space="PSUM") as ep_pool,
            tc.tile_pool(name="scp", bufs=2, space="PSUM") as scp_pool,
            tc.tile_pool(name="ebf", bufs=9) as e_pool,
            tc.tile_pool(name="pbf", bufs=4) as p_pool,
            tc.tile_pool(name="ttrv", bufs=3) as ttrv_pool,
            tc.tile_pool(name="prod", bufs=3) as prod_pool,
            tc.tile_pool(name="ttra", bufs=3) as ttra_pool,
        ):
            # ---- constants; fp8 stream (we8 + enc8) rides the scalar(ACT)
            # ring, bf16 ko-halves ride sync(SP) and Pool SWDGE rings.
            # we8 lands in per-sko pieces so the first matmul's weights
            # arrive first ----
            we8_sb = const.tile([128, SKO, 2, MH, 128], f8)
            for sko in range(SKO):
                nc.scalar.dma_start(we8_sb[:, sko], we8_d[:, sko])
            wo_sb = const.tile([128, MH, 128], bf16)
            nc.sync.dma_start(wo_sb, wo_d[:])
            bias_sb = const.tile([128, MH, B_LOC], f32)
            nc.sync.dma_start(bias_sb, bias_d[:])

            # ---- PE warmup: junk matmuls lift the HAM clock gate while the
            # first chunk's DMA is in flight ----
            warm_sb = const.tile([128, 128], bf16, name="warm_sb")
            nc.vector.memset(warm_sb, 0.0)
            warm_ps = scp_pool.tile([128, SP], f32, name="sc_ep")
            for _ in range(n_warm):
                nc.tensor.matmul(
                    warm_ps[:, :128], warm_sb, warm_sb, start=True, stop=True
                )

            # ---- accumulators (each column written exactly once) ----
            l_parts = const.tile([128, n_idx], f32)
            ctxv = const.tile([128, 4, n_idx], f32)  # ko 0-3
            ctxp = const.tile([128, 4, n_idx], f32)  # ko 4-7
            outf = const.tile([128, B_LOC, KO + 1], f32)

            # ---- pair epilogue: scores -> exp -> context accumulation.
            # Emitted one pair late so PE runs [ep(i)][sc(i-1)][ep(i+1)]. ----
            def emit_exp_ctx(b, slot, encb_lo, encb_hi, sc, cols, act_kos=ACT_KOS):
                w = cols.stop - cols.start
                p_b = p_pool.tile([128, w], bf16, name=f"pb{w}")
                nc.scalar.activation(
                    p_b, sc[:, cols], AF.Exp, accum_out=l_parts[:, slot : slot + 1]
                )
                for ko in range(KO):
                    encb_b = encb_lo if ko < 4 else encb_hi
                    acc = ctxv if ko < 4 else ctxp
                    acc_ap = acc[:, ko % 4, slot : slot + 1]
                    if ko < KO - act_kos:
                        scr = ttrv_pool.tile([128, w], bf16, name=f"scr{w}")
                        nc.vector.scalar_tensor_tensor(
                            out=scr,
                            in0=encb_b[:, ko % 4, cols],
                            scalar=1.0,
                            in1=p_b,
                            op0=ALU.mult,
                            op1=ALU.mult,
                            accum_out=acc_ap,
                        )
                    else:
                        prod = prod_pool.tile([128, w], bf16, name=f"prod{w}")
                        nc.vector.tensor_mul(prod, encb_b[:, ko % 4, cols], p_b)
                        scr = ttra_pool.tile([128, w], bf16, name=f"scrA{w}")
                        nc.scalar.activation(scr, prod, AF.Copy, accum_out=acc_ap)

            def emit_finalize(b):
                hi = (b + 1) * n_pairs + (n_tail_parts - 1 if b == B_LOC - 1 else 0)
                sl = slice(b * n_pairs, hi)
                nc.vector.reduce_sum(outf[:, b, 0:4], ctxv[:, :, sl], axis=X)
                nc.vector.reduce_sum(outf[:, b, 4:8], ctxp[:, :, sl], axis=X)
                nc.vector.reduce_sum(outf[:, b, KO : KO + 1], l_parts[:, sl], axis=X)

            def emit_epilogue(b, idx, encb_lo, encb_hi, e_tiles):
                last = idx == B_LOC * n_pairs - 1
                sc = scp_pool.tile([128, SP], f32, name="sc_ep")
                if last:
                    # final pair: column quarters pipeline the serial
                    # scores->exp->context chain at the kernel tail; all-out
                    # ACT offload splits the ctx work evenly with DVE
                    q = SP // n_tail_parts
                    for i in range(n_tail_parts):
                        cols = slice(i * q, (i + 1) * q)
                        for m in range(MH):
                            nc.tensor.matmul(
                                sc[:, cols],
                                wo_sb[:, m, :],
                                e_tiles[m][:, cols],
                                start=(m == 0),
                                stop=(m == MH - 1),
                            )
                        emit_exp_ctx(b, idx + i, encb_lo, encb_hi, sc, cols, act_kos=4)
                else:
                    for h in range(2):
                        cols = slice(h * 512, (h + 1) * 512)
                        for m in range(MH):
                            nc.tensor.matmul(
                                sc[:, cols],
                                wo_sb[:, m, :],
                                e_tiles[m][:, cols],
                                start=(m == 0),
                                stop=(m == MH - 1),
                            )
                    emit_exp_ctx(b, idx, encb_lo, encb_hi, sc, slice(0, SP))
                if idx % n_pairs == n_pairs - 1:
                    emit_finalize(b)

            # ---- main loop (epilogue software-pipelined by one pair) ----
            pending = None
            for b in range(B_LOC):
                for pr in range(n_pairs):
                    idx = b * n_pairs + pr
                    enc8_b = enc8_pool.tile([128, 2, SKO, 2, 512], f8)
                    if idx == 0:
                        # land pair 0 in (sko, h) pieces in matmul consumption
                        # order so the first ep matmul starts ASAP
                        for sko in range(SKO):
                            for h in range(2):
                                nc.scalar.dma_start(
                                    enc8_b[:, h, sko], enc8_d[b, pr, :, h, sko]
                                )
                    else:
                        nc.scalar.dma_start(enc8_b, enc8_d[b, pr])
                    encb_lo = encb_pool.tile([128, KO // 2, SP], bf16, name="enclo")
                    nc.sync.dma_start(encb_lo, enclo_d[b, pr])
                    encb_hi = encb_pool.tile([128, KO // 2, SP], bf16, name="enchi")
                    nc.gpsimd.dma_start(encb_hi, enchi_d[b, pr])
                    e_tiles = []
                    for m in range(MH):
                        ep = ep_pool.tile([128, SP], f32)
                        for sko in range(SKO):
                            for h in range(2):  # same weights for both halves
                                nc.tensor.matmul(
                                    ep[:, h * 512 : (h + 1) * 512],
                                    we8_sb[:, sko, :, m, :],
                                    enc8_b[:, h, sko, :, :],
                                    start=(sko == 0),
                                    stop=(sko == SKO - 1),
                                    perf_mode=DR,
                                )
                        e_m = e_pool.tile([128, SP], bf16)
                        nc.scalar.activation(
                            e_m,
                            ep,
                            AF.Tanh,
                            bias=bias_sb[:, m, b : b + 1],
                            scale=1.0 / WSCALE,
                        )
                        e_tiles.append(e_m)

                    if pending is not None:
                        emit_epilogue(*pending)
                    pending = (b, idx, encb_lo, encb_hi, e_tiles)
            emit_epilogue(*pending)
            nc.sync.dma_start(out_d[:], outf)

    nc.compile()
    return nc


def _host_prep(encoder_hiddens, decoder_hidden, We, be, Wd, bd, Wo):
    import ml_dtypes

    bf16 = ml_dtypes.bfloat16
    f8 = ml_dtypes.float8_e4m3fn
    n_pairs = S // SP

    enc = np.asarray(encoder_hiddens, dtype=np.float32)
    dec = np.asarray(decoder_hidden, dtype=np.float32)
    We_h = np.asarray(We, dtype=np.float32)
    Wd_h = np.asarray(Wd, dtype=np.float32)
    Wo_h = np.asarray(Wo, dtype=np.float32).reshape(-1)
    be_h = np.asarray(be, dtype=np.float32)
    bd_h = np.asarray(bd, dtype=np.float32)

    # weights / biases (shared across cores)
    we8 = np.ascontiguousarray(
        (We_h * WSCALE).reshape(SKO, 2, 128, MH, 128).transpose(2, 0, 1, 3, 4)
    ).astype(f8)
    wo = np.ascontiguousarray(
        np.broadcast_to(Wo_h.reshape(MH, 128).T[:, :, None], (128, MH, 128))
    ).astype(bf16)
    dp = dec @ Wd_h + (be_h + bd_h)  # [B, H] decoder projection + biases on host
    # bias[p, m, b] = dp[b, m*128+p]
    bias_all = np.ascontiguousarray(dp.reshape(B, MH, 128).transpose(2, 1, 0))

    in_maps = []
    for c in range(NCORES):
        b0 = c * B_LOC
        enc_c = enc[b0 : b0 + B_LOC]  # [B_loc, S, E]
        # bf16: [b, pr, p, ko, s] with e = ko*128+p, s in 0..1023
        encb = np.ascontiguousarray(
            enc_c.reshape(B_LOC, n_pairs, SP, KO, 128).transpose(0, 1, 4, 3, 2)
        ).astype(bf16)
        enclo = np.ascontiguousarray(encb[:, :, :, : KO // 2])
        enchi = np.ascontiguousarray(encb[:, :, :, KO // 2 :])
        # fp8: [b, pr, p, h, sko, j, s] with e = sko*256 + j*128 + p, s in 0..511
        enc8 = np.ascontiguousarray(
            enc_c.reshape(B_LOC, n_pairs, 2, 512, SKO, 2, 128).transpose(
                0, 1, 6, 2, 4, 5, 3
            )
        ).astype(f8)
        in_maps.append(
            {
                "enc8": enc8,
                "enclo": enclo,
                "enchi": enchi,
                "we8": we8,
                "wo": wo,
                "bias": np.ascontiguousarray(bias_all[:, :, b0 : b0 + B_LOC]),
            }
        )
    return in_maps


def _run(inputs, trace=False, **spmd_kwargs):
    from concourse.bass_utils import run_bass_kernel_spmd

    spmd_kwargs.pop("cast_dma", None)
    in_maps = _host_prep(
        inputs["encoder_hiddens"],
        inputs["decoder_hidden"],
        inputs["We"],
        inputs["be"],
        inputs["Wd"],
        inputs["bd"],
        inputs["Wo"],
    )
    nc = build_graph()
    res = run_bass_kernel_spmd(
        nc, in_maps, core_ids=list(range(NCORES)), trace=trace, **spmd_kwargs
    )
    outs = []
    for c in range(NCORES):
        arr = np.asarray(res.results[c]["out"], dtype=np.float64)  # [128, B_loc, KO+1]
        ctx = arr[:, :, :KO].transpose(1, 2, 0).reshape(B_LOC, E_ENC)  # e = ko*128+p
        l = arr[0, :, KO]  # identical across partitions
        outs.append(ctx / l[:, None])
    return np.ascontiguousarray(np.concatenate(outs, axis=0), dtype=np.float32), res


def kernel(**inputs):
    # One retry: a previously-crashed tenant can leave a core transiently
    # "unrecoverable" (or returning NaN) for the first NEFF execution; the
    # state clears on the next attempt. A retry is free when healthy.
    last_exc = None
    out = None
    for _ in range(2):
        try:
            out, _ = _run(inputs, trace=False)
        except Exception as exc:  # noqa: BLE001 - device transients
            last_exc = exc
            continue
        if np.isfinite(out).all():
            return out
    if out is None and last_exc is not None:
        raise last_exc
    return out
